# revision 5
# baseline (speedup 1.0000x reference)
"""v7 Bass kernel for nn_AlphaChebyshevProjection — 2-lane, friction-free.

Math: x0 = 0 exactly; q_i = (A_i @ x)/b_i (signed), maxq = max(max q, 1.0),
out = relu(x) * (1/maxq).

Lanes:
  a-rows [0, NA):  Pool tensor_tensor mult -> ACT per-row Copy+accum ->
                   one merged DVE q-stt. Early stream slots; small count
                   sized to ACT's serial 439ns/row budget.
  v-rows [NA, 85): DVE per-row fused scalar_tensor_tensor
                   (scr = (A_row*rb_row)*x, accum q_row) — one pass/row.
DVE consumes v rows slower than the DMA delivers them, so after the first
chunk lands its queue never empties: zero mid-stream stalls by
construction. Pool/ACT handle a small early block and finish before DVE
drains. Tail: merged a-q + pre-max (filler before the closing chunks) ->
final max -> reciprocal -> rx * alpha -> out store (no completion sem;
the queue drain covers it). b loads via Pool SWDGE before x; x first on
the HWDGE queue. Walrus sync rules: standalone wait_ge, counting sems for
same-engine RAW hazards (DVE does not self-serialize RAW on hw).
"""

import numpy as np

import concourse.bass as bass
from concourse import mybir
from concourse.bass_utils import run_bass_kernel_spmd

B, S, M, N = 64, 16, 85, 80
NCORES = 8
P = (B * S) // NCORES
FP32 = mybir.dt.float32

# stream-ordered (rows, kind); 'a' rows map to [0, NA), 'v' to [NA, M)
SCHED = [
    (8, "v"), (6, "a"), (8, "v"), (6, "a"), (8, "v"), (6, "a"), (8, "v"),
    (8, "v"), (8, "v"), (8, "v"), (7, "v"), (2, "v"), (2, "v"),
]
assert sum(w for w, _ in SCHED) == M
AQ_SLOT = -3   # insert merged a-q + pre-max before this v-chunk (from end)


def build_nc(sched=None, aq_slot=AQ_SLOT):
    sched = SCHED if sched is None else sched
    assert sum(w for w, _ in sched) == M
    NA = sum(w for w, k in sched if k == "a")
    K = len(sched)

    cur = {"a": 0, "v": NA}
    rng = []
    for w, kind in sched:
        rng.append((cur[kind], cur[kind] + w))
        cur[kind] += w
    assert cur["v"] == M

    nc = bass.Bass("TRN2", monotonic_sem_count=0)
    A_ext = nc.dram_tensor("A", [P, M, N], FP32, kind="ExternalInput")
    x_ext = nc.dram_tensor("x_hat", [P, N], FP32, kind="ExternalInput")
    b_ext = nc.dram_tensor("b", [P, M], FP32, kind="ExternalInput")
    out_ext = nc.dram_tensor("out", [P, N], FP32, kind="ExternalOutput")

    Alu = mybir.AluOpType
    Ax = mybir.AxisListType
    Act = mybir.ActivationFunctionType

    x_t = nc.alloc_sbuf_tensor("x_t", [P, N], FP32)
    rx = nc.alloc_sbuf_tensor("rx", [P, N], FP32)
    b_t = nc.alloc_sbuf_tensor("b_t", [P, M], FP32)
    rb = nc.alloc_sbuf_tensor("rb", [P, M], FP32)
    Ad = nc.alloc_sbuf_tensor("Ad", [P, M], FP32)
    # q: [0..M) rows, [M] seed 1.0, [M+1] pre-max cell
    q_t = nc.alloc_sbuf_tensor("q_t", [P, M + 2], FP32)
    a_ts = [nc.alloc_sbuf_tensor(f"a_{k}", [P, w, N], FP32)
            for k, (w, _) in enumerate(sched)]
    p_ts = [nc.alloc_sbuf_tensor(f"p_{k}", [P, w, N], FP32)
            if kind == "a" else None
            for k, (w, kind) in enumerate(sched)]
    scr = nc.alloc_sbuf_tensor("scr", [P, N], FP32)
    maxq = nc.alloc_sbuf_tensor("maxq", [P, 1], FP32)
    alpha = nc.alloc_sbuf_tensor("alpha", [P, 1], FP32)
    out_t = nc.alloc_sbuf_tensor("out_t", [P, N], FP32)
    warm = nc.alloc_sbuf_tensor("warm", [P, 1], FP32)

    def bc(t, w):
        ap = t[:, :]
        return bass.AP(tensor=ap.tensor, offset=ap.offset,
                       ap=[list(ap.ap[0]), [0, w], list(ap.ap[1])])

    with (
        nc.semaphore("s_x") as s_x,
        nc.semaphore("s_b") as s_b,
        nc.semaphore("s_g") as s_g,
        nc.semaphore("s_gm") as s_gm,
        nc.semaphore("s_act") as s_act,
        nc.semaphore("s_sc") as s_sc,
        nc.semaphore("s_d") as s_d,
        nc.semaphore("s_v") as s_v,
        nc.semaphore("s_out") as s_out,
        nc.Block() as block,
    ):
        s_c = [nc.ctx.enter_context(nc.semaphore(f"s_c{k}")) for k in range(K)]

        a_chunks = [k for k, (w, kind) in enumerate(sched) if kind == "a"]
        v_chunks = [k for k, (w, kind) in enumerate(sched) if kind == "v"]
        gc = {k: i + 1 for i, k in enumerate(a_chunks)}
        act_done = {}
        ar = 0
        for k in a_chunks:
            ar += sched[k][0]
            act_done[k] = ar
        n_act = ar

        @block.sync
        def _(sync):
            sync.dma_start(out=x_t[:, :], in_=x_ext[:, :]).then_inc(s_x, 16)
            for k, (w, _) in enumerate(sched):
                r0, r1 = rng[k]
                sync.dma_start(
                    out=a_ts[k][:, :, :], in_=A_ext[:, r0:r1, :]
                ).then_inc(s_c[k], 16)
            sync.wait_ge(s_v, 1)
            sync.dma_start(out=out_ext[:, :], in_=out_t[:, :]).then_inc(
                s_out, 16)

        @block.gpsimd
        def _(gp):
            gpe = nc.gpsimd
            gpe.dma_start(out=b_t[:, :], in_=b_ext[:, :]).then_inc(s_b, 16)
            gpe.memset(q_t[:, M:M + 1], 1.0).then_inc(s_gm, 1)
            gpe.memset(warm[:, :], 0.0).then_inc(s_gm, 1)
            gpe.wait_ge(s_x, 16)
            for k in a_chunks:
                w = sched[k][0]
                gpe.wait_ge(s_c[k], 16)
                gpe.tensor_tensor(
                    out=p_ts[k][:, :, :], in0=a_ts[k][:, :, :], in1=bc(x_t, w),
                    op=Alu.mult,
                ).then_inc(s_g, 1)

        @block.scalar
        def _(sc_):
            sc = nc.scalar
            sc.wait_ge(s_gm, 2)
            sc.activation(warm[:, :], warm[:, :], Act.Copy)
            sc.wait_ge(s_x, 16)
            sc.activation(rx[:, :], x_t[:, :], Act.Relu).then_inc(s_sc, 1)
            for k in a_chunks:
                w = sched[k][0]
                r0 = rng[k][0]
                sc.wait_ge(s_g, gc[k])
                for j in range(w):
                    sc.activation(
                        p_ts[k][:, j, :], p_ts[k][:, j, :], Act.Copy,
                        accum_out=Ad[:, r0 + j:r0 + j + 1],
                    ).then_inc(s_act, 1)

        @block.vector
        def _(vec):
            v = nc.vector
            nd = [0]

            def d(instr):
                instr.then_inc(s_d, 1)
                nd[0] += 1
                return nd[0]

            v.wait_ge(s_b, 16)
            i_rb = d(v.reciprocal(rb[:, :], b_t[:, :]))
            v.wait_ge(s_d, i_rb)

            aq_chunk = v_chunks[aq_slot]
            for k, (w, kind) in enumerate(sched):
                if kind != "v":
                    continue
                r0 = rng[k][0]
                if k == aq_chunk:
                    # merged q for the ACT block + pre-max filler
                    v.wait_ge(s_act, n_act)
                    i_aq = d(v.scalar_tensor_tensor(
                        out=q_t[:, 0:NA], in0=Ad[:, 0:NA], scalar=1.0,
                        in1=rb[:, 0:NA], op0=Alu.mult, op1=Alu.mult))
                    v.wait_ge(s_d, i_aq)
                    v.wait_ge(s_gm, 1)
                    d(v.tensor_reduce(
                        out=q_t[:, M + 1:M + 2], in_=q_t[:, 0:r0],
                        axis=Ax.X, op=Alu.max))
                v.wait_ge(s_c[k], 16)
                for j in range(w):
                    instr = v.scalar_tensor_tensor(
                        out=scr[:, :], in0=a_ts[k][:, j, :],
                        scalar=rb[:, r0 + j:r0 + j + 1], in1=x_t[:, :],
                        op0=Alu.mult, op1=Alu.mult,
                        accum_out=q_t[:, r0 + j:r0 + j + 1])
                    if j == w - 1:
                        d(instr)

            tail_lo = rng[aq_chunk][0]
            v.wait_ge(s_d, nd[0])
            i_mq = d(v.tensor_reduce(
                out=maxq[:, :], in_=q_t[:, tail_lo:M + 2], axis=Ax.X,
                op=Alu.max))
            v.wait_ge(s_d, i_mq)
            i_al = d(v.reciprocal(alpha[:, :], maxq[:, :]))
            v.wait_ge(s_d, i_al)
            v.wait_ge(s_sc, 1)
            v.tensor_scalar(
                out=out_t[:, :], in0=rx[:, :], scalar1=alpha[:, :],
                scalar2=None, op0=Alu.mult,
            ).then_inc(s_v, 1)

    return nc


def _run_spmd(x_hat, A, b, **kw):
    x = np.ascontiguousarray(np.asarray(x_hat, np.float32).reshape(B * S, N))
    Af = np.ascontiguousarray(np.asarray(A, np.float32).reshape(B * S, M, N))
    bf = np.ascontiguousarray(np.asarray(b, np.float32).reshape(B * S, M))
    in_maps = [
        {"A": Af[i * P:(i + 1) * P], "x_hat": x[i * P:(i + 1) * P],
         "b": bf[i * P:(i + 1) * P]}
        for i in range(NCORES)
    ]
    nc = build_nc()
    res = run_bass_kernel_spmd(nc, in_maps, core_ids=list(range(NCORES)), **kw)
    out = np.concatenate([res.results[i]["out"] for i in range(NCORES)], axis=0)
    return out.reshape(B, S, N).astype(np.float32), res


def kernel(x_hat, A, b):
    out, _ = _run_spmd(x_hat, A, b)
    return out


# revision 6
# speedup vs baseline: 1.0369x; 1.0369x over previous
"""v6 Bass kernel for nn_AlphaChebyshevProjection — list-scheduled DVE order.

Math: x0 = 0 exactly; q_i = (A_i @ x)/b_i (signed), maxq = max(max q, 1.0),
out = relu(x) * (1/maxq).

Lanes (contiguous row blocks):
  p: Pool tensor_tensor mult -> DVE chunk reduce; one merged q for [0,NP)
  a: Pool mult -> ACT per-row Copy+accum;        one merged q for [NP,NP+NA)
  w: DVE per-row STT (scalar=1.0) -> Ad;         one merged q for the block
  v: DVE per-row STT (scalar=rb_row, accum=q) — q direct (stream tail)
DVE's instruction order is LIST-SCHEDULED against a closed-form readiness
estimator (DMA stream positions, Pool mult queue, ACT serial chain) so no
item head-of-line blocks a ready one. Same walrus sync rules as before.
"""

import numpy as np

import concourse.bass as bass
from concourse import mybir
from concourse.bass_utils import run_bass_kernel_spmd

B, S, M, N = 64, 16, 85, 80
NCORES = 8
P = (B * S) // NCORES
FP32 = mybir.dt.float32

SCHED = [
    (8, "w"), (6, "a"), (6, "w"), (6, "a"), (8, "v"), (8, "p"), (8, "v"),
    (8, "p"), (7, "v"), (8, "p"), (2, "p"), (2, "v"), (4, "v"), (2, "v"),
    (2, "v"),
]
assert sum(w for w, _ in SCHED) == M

# --- cost-model constants for the readiness estimator (ns) ---
_T0 = 2500.0        # first A transfer start (after x)
_ROW = 113.75       # DMA ns/row
_SMALL = 227.5      # <=2-row transfer
_VIS = 900.0        # DMA sem prop
_POOL_ROW = 158.7
_POOL_LAUNCH = 95.0
_ACT_ROW = 439.0
_DVE_ROW = 143.7
_RED = lambda w: 83.3 * w + 60.0
_HOP = 250.0


def _plan(sched):
    """Estimated ready times for every DVE item; returns ordered item list."""
    K = len(sched)
    # transfer slots: x first, b (SWDGE) slots after chunk 0
    t = _T0
    vis = []
    for k, (w, kind) in enumerate(sched):
        dur = max(w * _ROW, _SMALL)
        t += dur
        vis.append(t + _VIS)
        if k == 0:
            t += 242.0  # b transfer slot
    b_vis = _T0 + max(sched[0][0] * _ROW, _SMALL) + 242.0 + _VIS
    x_vis = _T0 + 230.0 + _VIS

    pool_order = [k for k, (w, kind) in enumerate(sched) if kind == "a"]
    pool_order += [k for k, (w, kind) in enumerate(sched) if kind == "p"]
    pd = {}
    pt = x_vis
    for k in pool_order:
        w = sched[k][0]
        pt = max(pt, vis[k]) + w * _POOL_ROW + _POOL_LAUNCH
        pd[k] = pt
    at = 0.0
    act_end = {}
    for k, (w, kind) in enumerate(sched):
        if kind != "a":
            continue
        at = max(at, pd[k] + _HOP) + w * _ACT_ROW
        act_end[k] = at

    # DVE items: ("w",k) ("v",k) ("red",k) ready times
    items = []
    for k, (w, kind) in enumerate(sched):
        if kind in ("w", "v"):
            ready = vis[k] if kind == "w" else max(vis[k], b_vis + 200)
            items.append((ready, 0 if kind == "v" else 1, kind, k))
        elif kind == "p":
            items.append((pd[k] + _HOP, 2, "red", k))
    # sort by ready; ties prefer 'v' (tail critical)
    items.sort()
    return [(kind, k) for _, _, kind, k in items], act_end


def build_nc(sched=None):
    sched = SCHED if sched is None else sched
    assert sum(w for w, _ in sched) == M
    NP = sum(w for w, k in sched if k == "p")
    NA = sum(w for w, k in sched if k == "a")
    NW = sum(w for w, k in sched if k == "w")
    K = len(sched)

    cur = {"p": 0, "a": NP, "w": NP + NA, "v": NP + NA + NW}
    rng = []
    for w, kind in sched:
        rng.append((cur[kind], cur[kind] + w))
        cur[kind] += w
    assert cur["v"] == M

    dve_items, _ = _plan(sched)

    nc = bass.Bass("TRN2", monotonic_sem_count=0)
    A_ext = nc.dram_tensor("A", [P, M, N], FP32, kind="ExternalInput")
    x_ext = nc.dram_tensor("x_hat", [P, N], FP32, kind="ExternalInput")
    b_ext = nc.dram_tensor("b", [P, M], FP32, kind="ExternalInput")
    out_ext = nc.dram_tensor("out", [P, N], FP32, kind="ExternalOutput")

    Alu = mybir.AluOpType
    Ax = mybir.AxisListType
    Act = mybir.ActivationFunctionType

    x_t = nc.alloc_sbuf_tensor("x_t", [P, N], FP32)
    rx = nc.alloc_sbuf_tensor("rx", [P, N], FP32)
    b_t = nc.alloc_sbuf_tensor("b_t", [P, M], FP32)
    rb = nc.alloc_sbuf_tensor("rb", [P, M], FP32)
    Ad = nc.alloc_sbuf_tensor("Ad", [P, M], FP32)
    q_t = nc.alloc_sbuf_tensor("q_t", [P, M + 2], FP32)
    a_ts = [nc.alloc_sbuf_tensor(f"a_{k}", [P, w, N], FP32)
            for k, (w, _) in enumerate(sched)]
    p_ts = [nc.alloc_sbuf_tensor(f"p_{k}", [P, w, N], FP32)
            if kind in ("a", "p") else None
            for k, (w, kind) in enumerate(sched)]
    scr = nc.alloc_sbuf_tensor("scr", [P, N], FP32)
    maxq = nc.alloc_sbuf_tensor("maxq", [P, 1], FP32)
    alpha = nc.alloc_sbuf_tensor("alpha", [P, 1], FP32)
    out_t = nc.alloc_sbuf_tensor("out_t", [P, N], FP32)
    warm = nc.alloc_sbuf_tensor("warm", [P, 1], FP32)

    def bc(t, w):
        ap = t[:, :]
        return bass.AP(tensor=ap.tensor, offset=ap.offset,
                       ap=[list(ap.ap[0]), [0, w], list(ap.ap[1])])

    with (
        nc.semaphore("s_x") as s_x,
        nc.semaphore("s_b") as s_b,
        nc.semaphore("s_g") as s_g,
        nc.semaphore("s_gm") as s_gm,
        nc.semaphore("s_act") as s_act,
        nc.semaphore("s_sc") as s_sc,
        nc.semaphore("s_d") as s_d,
        nc.semaphore("s_v") as s_v,
        nc.semaphore("s_out") as s_out,
        nc.Block() as block,
    ):
        s_c = [nc.ctx.enter_context(nc.semaphore(f"s_c{k}")) for k in range(K)]

        pool_order = [k for k, (w, kind) in enumerate(sched) if kind == "a"]
        pool_order += [k for k, (w, kind) in enumerate(sched) if kind == "p"]
        gc = {k: i + 1 for i, k in enumerate(pool_order)}
        act_done = {}
        ar = 0
        for k, (w, kind) in enumerate(sched):
            if kind == "a":
                ar += w
                act_done[k] = ar
        n_act = ar

        @block.sync
        def _(sync):
            sync.dma_start(out=x_t[:, :], in_=x_ext[:, :]).then_inc(s_x, 16)
            for k, (w, _) in enumerate(sched):
                r0, r1 = rng[k]
                sync.dma_start(
                    out=a_ts[k][:, :, :], in_=A_ext[:, r0:r1, :]
                ).then_inc(s_c[k], 16)
            sync.wait_ge(s_v, 1)
            sync.dma_start(out=out_ext[:, :], in_=out_t[:, :]).then_inc(
                s_out, 16)

        @block.gpsimd
        def _(gp):
            gpe = nc.gpsimd
            gpe.dma_start(out=b_t[:, :], in_=b_ext[:, :]).then_inc(s_b, 16)
            gpe.memset(q_t[:, M:M + 1], 1.0).then_inc(s_gm, 1)
            gpe.memset(warm[:, :], 0.0).then_inc(s_gm, 1)
            gpe.wait_ge(s_x, 16)
            for k in pool_order:
                w = sched[k][0]
                gpe.wait_ge(s_c[k], 16)
                gpe.tensor_tensor(
                    out=p_ts[k][:, :, :], in0=a_ts[k][:, :, :], in1=bc(x_t, w),
                    op=Alu.mult,
                ).then_inc(s_g, 1)

        @block.scalar
        def _(sc_):
            sc = nc.scalar
            sc.wait_ge(s_gm, 2)
            sc.activation(warm[:, :], warm[:, :], Act.Copy)
            sc.wait_ge(s_x, 16)
            sc.activation(rx[:, :], x_t[:, :], Act.Relu).then_inc(s_sc, 1)
            for k, (w, kind) in enumerate(sched):
                if kind != "a":
                    continue
                r0 = rng[k][0]
                sc.wait_ge(s_g, gc[k])
                for j in range(w):
                    sc.activation(
                        p_ts[k][:, j, :], p_ts[k][:, j, :], Act.Copy,
                        accum_out=Ad[:, r0 + j:r0 + j + 1],
                    ).then_inc(s_act, 1)

        @block.vector
        def _(vec):
            v = nc.vector
            nd = [0]

            def d(instr):
                instr.then_inc(s_d, 1)
                nd[0] += 1
                return nd[0]

            def stt_rows(k, fused):
                w = sched[k][0]
                r0 = rng[k][0]
                v.wait_ge(s_c[k], 16)
                for j in range(w):
                    if fused:
                        instr = v.scalar_tensor_tensor(
                            out=scr[:, :], in0=a_ts[k][:, j, :],
                            scalar=rb[:, r0 + j:r0 + j + 1], in1=x_t[:, :],
                            op0=Alu.mult, op1=Alu.mult,
                            accum_out=q_t[:, r0 + j:r0 + j + 1])
                    else:
                        instr = v.scalar_tensor_tensor(
                            out=scr[:, :], in0=a_ts[k][:, j, :],
                            scalar=1.0, in1=x_t[:, :],
                            op0=Alu.mult, op1=Alu.mult,
                            accum_out=Ad[:, r0 + j:r0 + j + 1])
                    if j == w - 1:
                        d(instr)

            # emit in list-scheduled order; rb after the first item,
            # merged q's as soon as their deps are emitted + expected ready
            v_chunks = [k for k, (w, kind) in enumerate(sched)
                        if kind == "v"]
            lastv2 = v_chunks[-2]
            n_p = sum(1 for w, k in sched if k == "p")
            n_w = sum(1 for w, k in sched if k == "w")
            did_rb = False
            reds = 0
            ws = 0
            first = True
            for kind, k in dve_items:
                if first:
                    # rb as the very first (b lands early via SWDGE)
                    v.wait_ge(s_b, 16)
                    i_rb = d(v.reciprocal(rb[:, :], b_t[:, :]))
                    v.wait_ge(s_d, i_rb)
                    did_rb = True
                    first = False
                if k == lastv2:
                    # before the second-to-last v chunk: merged ACT q
                    v.wait_ge(s_act, n_act)
                    d(v.scalar_tensor_tensor(
                        out=q_t[:, NP:NP + NA], in0=Ad[:, NP:NP + NA],
                        scalar=1.0, in1=rb[:, NP:NP + NA],
                        op0=Alu.mult, op1=Alu.mult))
                if kind == "w":
                    stt_rows(k, fused=False)
                    ws += 1
                    if ws == n_w:
                        i_pre = nd[0]
                        v.wait_ge(s_d, i_pre)
                        d(v.scalar_tensor_tensor(
                            out=q_t[:, NP + NA:NP + NA + NW],
                            in0=Ad[:, NP + NA:NP + NA + NW], scalar=1.0,
                            in1=rb[:, NP + NA:NP + NA + NW],
                            op0=Alu.mult, op1=Alu.mult))
                elif kind == "v":
                    stt_rows(k, fused=True)
                elif kind == "red":
                    r0, r1 = rng[k]
                    v.wait_ge(s_g, gc[k])
                    d(v.tensor_reduce(
                        out=Ad[:, r0:r1], in_=p_ts[k][:, :, :],
                        axis=Ax.X, op=Alu.add))
                    reds += 1
                    if reds == n_p:
                        i_red = nd[0]
                        v.wait_ge(s_d, i_red)
                        d(v.scalar_tensor_tensor(
                            out=q_t[:, 0:NP], in0=Ad[:, 0:NP], scalar=1.0,
                            in1=rb[:, 0:NP], op0=Alu.mult, op1=Alu.mult))

            v.wait_ge(s_gm, 1)
            v.wait_ge(s_d, nd[0])
            i_mq = d(v.tensor_reduce(
                out=maxq[:, :], in_=q_t[:, 0:M + 1], axis=Ax.X,
                op=Alu.max))
            v.wait_ge(s_d, i_mq)
            i_al = d(v.reciprocal(alpha[:, :], maxq[:, :]))
            v.wait_ge(s_d, i_al)
            v.wait_ge(s_sc, 1)
            v.tensor_scalar(
                out=out_t[:, :], in0=rx[:, :], scalar1=alpha[:, :],
                scalar2=None, op0=Alu.mult,
            ).then_inc(s_v, 1)

    return nc


def _run_spmd(x_hat, A, b, **kw):
    x = np.ascontiguousarray(np.asarray(x_hat, np.float32).reshape(B * S, N))
    Af = np.ascontiguousarray(np.asarray(A, np.float32).reshape(B * S, M, N))
    bf = np.ascontiguousarray(np.asarray(b, np.float32).reshape(B * S, M))
    in_maps = [
        {"A": Af[i * P:(i + 1) * P], "x_hat": x[i * P:(i + 1) * P],
         "b": bf[i * P:(i + 1) * P]}
        for i in range(NCORES)
    ]
    nc = build_nc()
    res = run_bass_kernel_spmd(nc, in_maps, core_ids=list(range(NCORES)), **kw)
    out = np.concatenate([res.results[i]["out"] for i in range(NCORES)], axis=0)
    return out.reshape(B, S, N).astype(np.float32), res


def kernel(x_hat, A, b):
    out, _ = _run_spmd(x_hat, A, b)
    return out
